# revision 21
# baseline (speedup 1.0000x reference)
"""GazeLoss Trainium2 kernel.

Strategy (pure data parallel over batch, 8 items per core):
  - The reference crops two 32x32 eye patches (bilinear grid_sample) from
    pred and target and takes mean|pl - tl|.  Bilinear sampling is linear
    in the image, and the sampling grid is separable, so the patch equals
    Ry @ img @ Rx^T with Ry/Rx (32, 512) sparse interpolation matrices
    that depend only on the landmarks.
  - Host: builds Ry/Rx (bf16) from landmarks (tiny), shards batch 8-way.
  - Device (per batch item, per channel):
      stage 1:  B = [RyL;RyR] @ P - [RyL;RyR] @ T      (8 bf16 matmuls,
                K=128 row-chunks of P/T streamed straight from HBM with
                f32->bf16 cast during DMA; accumulated in PSUM f32)
      stage 2:  C = B @ [RxL|RxR]^T  via PE transpose of B then 4 matmuls
      reduce:   acc[:, j] = sum_w |C| over the two valid eye quadrants
  - Host: sums the 8 per-core (64,1) partials -> loss.
"""

import numpy as np
import ml_dtypes

# ---- problem constants (hardcoded; kernel.py must be self-contained) ----
B, C, H, W = 64, 3, 512, 512
S = 32
PAD = 0.3
LEFT_IDX = np.arange(36, 42)
RIGHT_IDX = np.arange(42, 48)
N_CORES = 8
BPC = B // N_CORES  # batch items per core

BF16 = ml_dtypes.bfloat16

# ---------------------------------------------------------------------------
# Host-side: landmark -> interpolation matrices (mirrors reference f32 math)
# ---------------------------------------------------------------------------


def _eye_bbox(lm, idx):
    pts = lm[:, idx, :]
    x_min = pts[:, :, 0].min(axis=1)
    x_max = pts[:, :, 0].max(axis=1)
    y_min = pts[:, :, 1].min(axis=1)
    y_max = pts[:, :, 1].max(axis=1)
    w = x_max - x_min
    h = y_max - y_min
    return (x_min - w * np.float32(PAD), y_min - h * np.float32(PAD),
            x_max + w * np.float32(PAD), y_max + h * np.float32(PAD))


def _grid_1d(x1, y1, x2, y2):
    """f32 replica of reference grid; returns px (B,S), py (B,S)."""
    bx1 = np.clip(x1, 0.0, W - 1.0).astype(np.float32)
    by1 = np.clip(y1, 0.0, H - 1.0).astype(np.float32)
    bx2 = np.clip(x2, 0.0, W - 1.0).astype(np.float32)
    by2 = np.clip(y2, 0.0, H - 1.0).astype(np.float32)
    degenerate = (bx2 - bx1 < 1.0) | (by2 - by1 < 1.0)
    xn0 = bx1 / (W - 1) * np.float32(2.0) - np.float32(1.0)
    xn1 = bx2 / (W - 1) * np.float32(2.0) - np.float32(1.0)
    yn0 = by1 / (H - 1) * np.float32(2.0) - np.float32(1.0)
    yn1 = by2 / (H - 1) * np.float32(2.0) - np.float32(1.0)
    t = np.arange(S, dtype=np.float32) / np.float32(S - 1)
    xs = xn0[:, None] + (xn1 - xn0)[:, None] * t
    ys = yn0[:, None] + (yn1 - yn0)[:, None] * t
    xs[degenerate] = 0.0
    ys[degenerate] = 0.0
    px = np.clip((xs + np.float32(1.0)) * np.float32(0.5) * (W - 1), 0.0, W - 1.0)
    py = np.clip((ys + np.float32(1.0)) * np.float32(0.5) * (H - 1), 0.0, H - 1.0)
    return px.astype(np.float32), py.astype(np.float32)


def _interp_matrix(p):
    """p: (nb, S) coords -> M (nb, 512, S): M[b, i, t] = weight of line i
    at grid point t (bilinear, border-clamped)."""
    nb = p.shape[0]
    M = np.zeros((nb, 512, S), dtype=np.float32)
    p0 = np.floor(p)
    w = (p - p0).astype(np.float32)
    i0 = np.clip(p0, 0, 511).astype(np.int64)
    i1 = np.clip(p0 + 1, 0, 511).astype(np.int64)
    bidx = np.arange(nb)[:, None]
    tidx = np.broadcast_to(np.arange(S)[None, :], (nb, S))
    np.add.at(M, (bidx, i0, tidx), 1 - w)
    np.add.at(M, (bidx, i1, tidx), w)
    return M


def _host_prepare(landmarks):
    """Build per-batch Ry/Rx device layouts.

    Returns (ry, rx): each (B, 128, 4, 64) float32 where
      ry[g, p, hi, m]  = RyLR[g][4*p + hi, m]   (row-block-permuted lhsT)
      rx[g, p, k,  m]  = RxLR[g][128*k + p, m]  (w-chunked stage-2 rhs)
    with RyLR = [RyL | RyR] (512, 64), m<32 left eye, m>=32 right eye.
    """
    rys, rxs = [], []
    for idx in (LEFT_IDX, RIGHT_IDX):
        x1, y1, x2, y2 = _eye_bbox(landmarks, idx)
        px, py = _grid_1d(x1, y1, x2, y2)
        rys.append(_interp_matrix(py))  # (B, 512, 32)
        rxs.append(_interp_matrix(px))
    ry_lr = np.concatenate(rys, axis=2)  # (B, 512, 64)
    rx_lr = np.concatenate(rxs, axis=2)
    ry = ry_lr.reshape(B, 128, 4, 64)                    # row = 4p + hi
    rx = rx_lr.reshape(B, 4, 128, 64).transpose(0, 2, 1, 3)  # (B,128,4,64)
    return ry, rx


# ---------------------------------------------------------------------------
# Device kernel
# ---------------------------------------------------------------------------

_CACHE = {}


def _build_nc():
    import concourse.bass as bass
    import concourse.bacc as bacc
    import concourse.tile as tile
    from concourse import mybir

    f32 = mybir.dt.float32
    bf16 = mybir.dt.bfloat16

    nc = bacc.Bacc("TRN2", target_bir_lowering=False, debug=False)

    pred_h = nc.dram_tensor("pred", [BPC, C, H, W], f32, kind="ExternalInput")
    targ_h = nc.dram_tensor("target", [BPC, C, H, W], f32, kind="ExternalInput")
    ryp_h = nc.dram_tensor("ryp", [128, BPC * 4 * 64], bf16, kind="ExternalInput")
    rx_h = nc.dram_tensor("rx", [128, BPC * 4 * 64], bf16, kind="ExternalInput")
    out_h = nc.dram_tensor("out", [64, BPC * C], f32, kind="ExternalOutput")
    ident_h = nc.inline_tensor(
        np.eye(64, dtype=np.float32).astype(BF16), name="ident64"
    )

    pred = pred_h.ap()
    targ = targ_h.ap()

    with tile.TileContext(nc) as tc:
        with (
            tc.tile_pool(name="consts", bufs=1) as consts,
            tc.tile_pool(name="img", bufs=5) as img_pool,
            tc.tile_pool(name="bsb", bufs=3) as bsb_pool,
            tc.tile_pool(name="btsb", bufs=4) as btsb_pool,
            tc.tile_pool(name="accp", bufs=1) as acc_pool,
            tc.tile_pool(name="psB", bufs=2, space=bass.MemorySpace.PSUM) as psB,
            tc.tile_pool(name="psT", bufs=2, space=bass.MemorySpace.PSUM) as psT,
            tc.tile_pool(name="psC", bufs=2, space=bass.MemorySpace.PSUM) as psC,
        ):
            ry_t = consts.tile([128, BPC * 4 * 64], bf16, tag="ryp")
            nc.sync.dma_start(ry_t[:], ryp_h.ap()[:])
            rx_t = consts.tile([128, BPC * 4 * 64], bf16, tag="rx")
            nc.sync.dma_start(rx_t[:], rx_h.ap()[:])
            ident_t = consts.tile([64, 64], bf16, tag="ident")
            nc.sync.dma_start(ident_t[:], ident_h.ap()[:])
            acc = acc_pool.tile([64, BPC * C], f32, tag="acc")

            for b in range(BPC):
                # image tiles: partition p holds rows 4p..4p+3 (per channel);
                # f32->bf16 cast during DMA (SWDGE), 8 KiB src per descriptor
                p_tile = img_pool.tile([128, C * 4, 512], bf16, tag="p")
                t_tile = img_pool.tile([128, C * 4, 512], bf16, tag="t")
                for c in range(C):
                    src = pred[b, c].rearrange("(p h) w -> p (h w)", h=4)
                    nc.gpsimd.dma_start(p_tile[:, c * 4:(c + 1) * 4, :], src)
                    src = targ[b, c].rearrange("(p h) w -> p (h w)", h=4)
                    nc.gpsimd.dma_start(t_tile[:, c * 4:(c + 1) * 4, :], src)

                # D = P - T in place on the (otherwise idle) vector engine:
                # halves the stage-1 matmul + ldweights load on TensorE.
                # hi-slice granularity so each stage-1 matmul waits only on
                # its own 512-col slice — shortens the end-of-kernel drain
                for c in range(C):
                    for hi in range(4):
                        nc.vector.tensor_sub(
                            p_tile[:, c * 4 + hi, :],
                            p_tile[:, c * 4 + hi, :],
                            t_tile[:, c * 4 + hi, :],
                        )

                for c in range(C):
                    # stage 1: B = Ry@(P-T)  (accumulate 4 matmuls)
                    b_ps = psB.tile([64, 512], f32, tag="B")
                    for hi in range(4):
                        lhs = ry_t[:, (b * 4 + hi) * 64:(b * 4 + hi + 1) * 64]
                        nc.tensor.matmul(
                            b_ps[:], lhs, p_tile[:, c * 4 + hi, :],
                            start=(hi == 0), stop=(hi == 3),
                        )
                    # PSUM f32 -> SBUF bf16
                    b_sb = bsb_pool.tile([64, 512], bf16, tag="Bs")
                    nc.scalar.copy(b_sb[:], b_ps[:])

                    # stage 2: C = B @ RxLR ; via PE transpose of B chunks
                    c_ps = psC.tile([64, 64], f32, tag="C")
                    for k in range(4):
                        bt_ps = psT.tile([128, 64], bf16, tag="BT")
                        nc.tensor.transpose(
                            bt_ps[:], b_sb[:, k * 128:(k + 1) * 128], ident_t[:]
                        )
                        bt_sb = btsb_pool.tile([128, 64], bf16, tag="BTs")
                        nc.scalar.copy(bt_sb[:], bt_ps[:])
                        nc.tensor.matmul(
                            c_ps[:], bt_sb[:],
                            rx_t[:, (b * 4 + k) * 64:(b * 4 + k + 1) * 64],
                            start=(k == 0), stop=(k == 3),
                        )

                    # abs + row-sum of the two valid quadrants
                    j = b * C + c
                    nc.vector.reduce_sum(
                        out=acc[0:32, j:j + 1], in_=c_ps[0:32, 0:32],
                        axis=mybir.AxisListType.X, apply_absolute_value=True,
                    )
                    nc.vector.reduce_sum(
                        out=acc[32:64, j:j + 1], in_=c_ps[32:64, 32:64],
                        axis=mybir.AxisListType.X, apply_absolute_value=True,
                    )

            # ship per-(b,c) partials directly; host does the final 24-col
            # sum — keeps the last reduce off the end-of-kernel critical path
            nc.sync.dma_start(out_h.ap()[:], acc[:])

    nc.compile()
    return nc


def _get_nc():
    if "nc" not in _CACHE:
        _CACHE["nc"] = _build_nc()
    return _CACHE["nc"]


def _make_in_maps(pred, target, landmarks):
    ry, rx = _host_prepare(landmarks)  # (B,128,4,64) f32 each
    ry_b = ry.astype(BF16)
    rx_b = rx.astype(BF16)
    in_maps = []
    for ci in range(N_CORES):
        sl = slice(ci * BPC, (ci + 1) * BPC)
        # device layout (128, BPC, 4, 64) -> flatten free dims
        in_maps.append({
            "pred": np.ascontiguousarray(pred[sl]),
            "target": np.ascontiguousarray(target[sl]),
            "ryp": np.ascontiguousarray(
                ry_b[sl].transpose(1, 0, 2, 3)).reshape(128, BPC * 4 * 64),
            "rx": np.ascontiguousarray(
                rx_b[sl].transpose(1, 0, 2, 3)).reshape(128, BPC * 4 * 64),
        })
    return in_maps


def _run(pred, target, landmarks, trace=False, trace_kwargs=None):
    from concourse.bass_utils import run_bass_kernel_spmd

    nc = _get_nc()
    in_maps = _make_in_maps(pred, target, landmarks)
    res = run_bass_kernel_spmd(
        nc, in_maps, core_ids=list(range(N_CORES)), trace=trace,
        **(trace_kwargs or {}),
    )
    sums_l = 0.0
    sums_r = 0.0
    for core_out in res.results:
        o = np.asarray(core_out["out"], dtype=np.float64)  # (64, BPC*C)
        sums_l += o[0:32, :].sum()
        sums_r += o[32:64, :].sum()
    n = B * C * S * S
    loss = np.float32((sums_l / n + sums_r / n) / 2.0)
    return loss, res


def kernel(pred, target, landmarks):
    pred = np.asarray(pred, dtype=np.float32)
    target = np.asarray(target, dtype=np.float32)
    landmarks = np.asarray(landmarks, dtype=np.float32)
    loss, _ = _run(pred, target, landmarks, trace=False)
    return np.asarray(loss, dtype=np.float32)
